# revision 19
# baseline (speedup 1.0000x reference)
"""Trainium2 Bass kernel for nn_HEMoETorch_43722767073393 (moe_routing).

Reference computation:
    h        = embed[x]                                  (N=4096, D=1024)
    h_fast   = relu(h @ fast_w1.T)
    scores   = exp(-max(||h-mu||^2, 0) / (2*sigma^2)) * charge     (N, 64)
    top_idx  = top_k(scores.mean(0), 8); top_w = scores[:, top_idx]
    slow_out = sum_k top_w[:,k] * (h @ expert_w[top_idx[k]].T)
    out      = (h_fast + 0.3 * slow_out) @ fast_w2.T     (N, 50257)

Numerical structure exploited: with D=1024, ||h - mu||^2 is ~1280 +- 60 for
every (token, expert) pair, so exp(-sq/8) < 1e-55 underflows to exactly 0.0
in fp32 for ALL pairs.  Hence top_w == 0 and slow_out == 0 *exactly* in the
fp32 reference, and the output is exactly relu(embed[x] @ W1^T) @ W2^T.
We verify this on the host (same fp32 underflow semantics); if it ever did
not hold we fall back to passing the host-computed h_merged.

Device strategy (8 NeuronCores, no collectives): 2D shard grid
T (token ways) x W (vocab ways) = 8.  Each core computes
  phase A: hf^T = relu(W1 @ h^T)  for its 4096/T tokens      (bf16)
  phase C: logits^T[vocab shard, token shard]                (f16 out)
Per-matmul cost on this stack is ~257 ns: 216 ns moving-operand stream
(512 cols @ 2.4 GHz) + ~41 ns serialized per-MM LDWEIGHTS that cannot
be amortized (tile_legalize emits 1:1 LDW:MM and hardware requires the
pairing — post-schedule LDW pruning passes CoreSim+compile but yields
garbage on HW).  Kernel time ~= MM count x 257 ns with DMA/evictions
fully hidden, so the shard grid minimizes total matmul count: T=8/W=1
gives 3208 MMs/core vs 3712 for pure vocab sharding (phase A is
replicated W times; larger T shrinks it).  Phase A also doubles as the
rep-boundary prefetch buffer: its inputs (ht/w1) free early each rep,
so the next rep's DMAs overlap phase C and the PE never goes idle.
Phase C's stationary operand (w2) is pre-quantized on the host to
fp8 e3m4 (4 mantissa bits) at a power-of-2 scale that the host
divides back out of the f16 outputs: FWL loads fp8 weights faster
than bf16, shrinking the per-MM LDWEIGHTS cost, and the weight DMA
halves.  Measured rel err 1.4243e-2 vs the 2e-2 gate (bf16
everywhere: 3.2e-3); host-side ml_dtypes simulation reproduces the
HW error to 4 decimal places.
"""

import numpy as np
import ml_dtypes

import concourse.bass as bass  # noqa: F401  (bass must import before bacc)
import concourse.mybir as mybir
import concourse.tile as tile
from concourse import bacc
from concourse.bass_utils import run_bass_kernel_spmd

BF16 = ml_dtypes.bfloat16
F16 = np.float16

N_CORES = 8
T_WAYS = 8            # token shard ways
W_WAYS = 1            # vocab shard ways
B, S = 4, 1024
N = B * S             # 4096 tokens total
D = 1024
V = 50257
JT = D // 128         # 8 contraction tiles

NT = N // T_WAYS      # tokens per core
NCH = NT // 512       # 512-token chunks per core
VB_TOT = (V + 127) // 128          # 393 vocab blocks total
VBC = (VB_TOT + W_WAYS - 1) // W_WAYS   # vocab blocks per core
VSC = VBC * 128       # padded vocab per core

SIGMA = 2.0
FAST_RATIO = 0.7
TOP_K = 8

F8E3 = mybir.dt.np(mybir.dt.float8e3)   # ml_dtypes.float8_e3m4, max 15.5
_SW = 256.0           # w2 pre-quantization scale (power of 2, host de-scaled)

_prog_cache: dict = {}


def build_program(with_fast: bool = True, reps: int = 1):
    """Per-core SPMD program for one (token, vocab) shard.

    with_fast=True : device computes hf^T = relu(W1 @ h^T), then logits.
    with_fast=False: input "ht" already holds h_merged^T; only the logits
                     matmul runs (host fallback path).
    """
    nc = bacc.Bacc("TRN2", target_bir_lowering=False, debug=False,
                   num_devices=N_CORES)
    bf = mybir.dt.bfloat16
    f16 = mybir.dt.float16
    f32 = mybir.dt.float32
    f8 = mybir.dt.float8e3

    ht_d = nc.dram_tensor("ht", [D, NT], bf, kind="ExternalInput").ap()
    if with_fast:
        w1t_d = nc.dram_tensor("w1t", [D, D], bf, kind="ExternalInput").ap()
    w2p_d = nc.dram_tensor("w2p", [VSC, D], f8, kind="ExternalInput").ap()
    out_d = nc.dram_tensor("out", [VSC, NT], f16, kind="ExternalOutput").ap()

    with tile.TileContext(nc) as tc:
        with (
            tc.tile_pool(name="persist", bufs=1) as persist,
            tc.tile_pool(name="w2s", bufs=8) as w2s,
            tc.tile_pool(name="ostage", bufs=8) as ostage,
            tc.tile_pool(name="psum", bufs=8, space="PSUM") as psum,
        ):
          with (tc.For_i(0, reps, 1) if reps > 1
                else __import__("contextlib").nullcontext()):
            # resident h^T tiles: partition = d (j-block), free = tokens.
            # Interleave w1/ht so phase A's j-th matmul can start as soon
            # as pair j has landed.
            ht = []
            w1 = []
            for j in range(JT):
                if with_fast:
                    t = persist.tile([128, D], bf, tag=f"w1_{j}",
                                     name=f"w1_{j}")
                    nc.sync.dma_start(t[:], w1t_d[j * 128:(j + 1) * 128, :])
                    w1.append(t)
                t = persist.tile([128, NT], bf, tag=f"ht{j}", name=f"ht{j}")
                nc.sync.dma_start(t[:], ht_d[j * 128:(j + 1) * 128, :])
                ht.append(t)

            if with_fast:
                # phase A: hf^T[i-block, :] = relu(sum_j W1^T[j,:]^T h^T[j,:])
                hf = [persist.tile([128, NT], bf, tag=f"hf{i}", name=f"hf{i}")
                      for i in range(JT)]
                for i in range(JT):
                    pss = [psum.tile([128, 512], f32, tag="ps",
                                     name=f"psA{i}_{n}")
                           for n in range(NCH)]
                    for j in range(JT):
                        for n in range(NCH):
                            nc.tensor.matmul(
                                pss[n][:],
                                w1[j][:, i * 128:(i + 1) * 128],
                                ht[j][:, n * 512:(n + 1) * 512],
                                start=(j == 0), stop=(j == JT - 1),
                            )
                    for n in range(NCH):
                        nc.scalar.activation(
                            hf[i][:, n * 512:(n + 1) * 512], pss[n][:],
                            mybir.ActivationFunctionType.Relu,
                        )
            else:
                hf = ht

            # phase C: out^T[vb-block, tokens] accumulating over d.
            # host pre-tiled w2p so each vb block is one contiguous
            # [128, D] DMA.
            for vb in range(VBC):
                w2c = w2s.tile([128, D], f8, tag="w2c")
                nc.sync.dma_start(w2c[:], w2p_d[vb * 128:(vb + 1) * 128, :])
                pss = [psum.tile([128, 512], f32, tag="ps",
                                 name=f"psC{vb}_{n}")
                       for n in range(NCH)]
                for j in range(JT):
                    for n in range(NCH):
                        nc.tensor.matmul(
                            pss[n][:],
                            w2c[:, j * 128:(j + 1) * 128],
                            hf[j][:, n * 512:(n + 1) * 512],
                            start=(j == 0), stop=(j == JT - 1),
                        )
                for n in range(NCH):
                    ot = ostage.tile([128, 512], f16, tag="ot")
                    nc.vector.tensor_copy(ot[:], pss[n][:])
                    nc.sync.dma_start(
                        out_d[vb * 128:(vb + 1) * 128,
                              n * 512:(n + 1) * 512],
                        ot[:],
                    )

    nc.compile()
    return nc


def _routing_host(x, embed, expert_mu, expert_charge):
    """fp32 host replica of the routing math (same underflow semantics as
    the jax fp32 reference).  Returns (top_idx, top_w, h)."""
    h = embed[x.reshape(-1)].astype(np.float32)                    # (N, D)
    sq = (
        np.sum(h * h, axis=1, keepdims=True)
        + np.sum(expert_mu * expert_mu, axis=1)[None, :]
        - 2.0 * (h @ expert_mu.T)
    ).astype(np.float32)
    kern = np.exp(-np.maximum(sq, 0.0) / np.float32(2.0 * SIGMA ** 2),
                  dtype=np.float32)
    scores = kern * expert_charge[None, :].astype(np.float32)
    mean = scores.mean(axis=0, dtype=np.float32)
    # jax.lax.top_k: descending by value, ties broken by lower index
    top_idx = np.lexsort((np.arange(mean.shape[0]), -mean))[:TOP_K]
    return top_idx, scores[:, top_idx], h


def prepare_inputs(x, embed, fast_w1, fast_w2, expert_mu, expert_w,
                   expert_charge):
    """Host-side shard prep. Returns (with_fast, in_maps)."""
    x = np.asarray(x).astype(np.int64).reshape(-1)
    embed = np.asarray(embed, dtype=np.float32)
    fast_w1 = np.asarray(fast_w1, dtype=np.float32)
    fast_w2 = np.asarray(fast_w2, dtype=np.float32)
    expert_mu = np.asarray(expert_mu, dtype=np.float32)
    expert_charge = np.asarray(expert_charge, dtype=np.float32)

    top_idx, top_w, h = _routing_host(x, embed, expert_mu, expert_charge)

    if not np.any(top_w):
        # expected path: slow branch is exactly zero
        with_fast = True
        hm = h
        w1t = np.ascontiguousarray(fast_w1.T).astype(BF16)         # (D, D)
    else:  # pragma: no cover - degenerate-input safety net
        with_fast = False
        expert_w = np.asarray(expert_w, dtype=np.float32)
        h_fast = np.maximum(h @ fast_w1.T, 0.0)
        slow = np.zeros_like(h_fast)
        for k in range(TOP_K):
            slow += top_w[:, k:k + 1] * (h @ expert_w[top_idx[k]].T)
        hm = h_fast + np.float32(1.0 - FAST_RATIO) * slow
        w1t = None

    hmt = np.ascontiguousarray(hm.T).astype(BF16)                  # (D, N)

    # vocab shards: pre-tile so each 128-row vocab block is one contiguous
    # [128, D] DMA: w2p[vb*128+p, j*128+vcol] = w2T[j*128+p, vb*128+vcol]
    # Weights are pre-quantized to e3m4 (4 mantissa bits) at a power-of-2
    # scale; the scale divides out on the host during output assembly.
    global _SW
    wmax = np.abs(fast_w2).max()
    _SW = float(2.0 ** np.floor(np.log2(15.0 / max(wmax, 1e-30))))
    w2tb = (fast_w2.T * np.float32(_SW)).astype(F8E3)              # (D, V)
    w2t_full = np.zeros((D, VSC * W_WAYS), dtype=F8E3)
    w2t_full[:, :V] = w2tb
    w2ps = []
    for w in range(W_WAYS):
        sh = w2t_full[:, w * VSC:(w + 1) * VSC]
        w2p = np.ascontiguousarray(
            sh.reshape(JT, 128, VBC, 128).transpose(2, 1, 0, 3)
        ).reshape(VSC, D)
        w2ps.append(w2p)

    in_maps = []
    for c in range(N_CORES):
        t, w = divmod(c, W_WAYS)
        m = {"ht": np.ascontiguousarray(hmt[:, t * NT:(t + 1) * NT]),
             "w2p": w2ps[w]}
        if with_fast:
            m["w1t"] = w1t
        in_maps.append(m)
    return with_fast, in_maps


def kernel(**inputs) -> np.ndarray:
    with_fast, in_maps = prepare_inputs(**inputs)
    key = with_fast
    if key not in _prog_cache:
        _prog_cache[key] = build_program(with_fast)
    nc = _prog_cache[key]
    res = run_bass_kernel_spmd(nc, in_maps, core_ids=list(range(N_CORES)))
    # per-core output is transposed logits (VSC, NT) scaled by _SW;
    # assemble (V, N) and divide the weight scale back out.
    inv = np.float32(1.0 / _SW)
    full_t = np.empty((V, N), dtype=np.float32)
    for c in range(N_CORES):
        t, w = divmod(c, W_WAYS)
        sh = res.results[c]["out"]                   # (VSC, NT) f16
        v0 = w * VSC
        v1 = min(V, v0 + VSC)
        if v1 > v0:
            full_t[v0:v1, t * NT:(t + 1) * NT] = sh[:v1 - v0] * inv
    return np.ascontiguousarray(full_t.T)
